# revision 24
# baseline (speedup 1.0000x reference)
"""Trainium2 Bass kernel for nn_Agent_lstm (root MLP -> LSTM scan -> critic).

Data-parallel over B=4096 envs across 8 NeuronCores (512 envs/core), with
feature-on-partition ("transposed") layout on-device.

v2 design (vs. the 4-matmul baseline):
  - fused K=128 scan matmul: state rows = [hid_t (0:64); h_{t-1} (64:128)],
    weights = [Wih; Whh] stacked -> 2 matmul instrs/step instead of 4.
  - sigmoid-only gates: tanh(g) = 2*sigmoid(2g) - 1 with the x2 folded into
    the g weight columns; the (2u-1) fixup runs on the otherwise-idle GPSIMD
    engine.  All four gates then take ONE strided ACTIVATE over both PSUM
    banks per step per group.
  - partition-packed DVE ops: [sf;si]*[cm;gt] -> [A;B] and
    [c';so]*[m;tanh_c] -> [cm';H] compute two 64-row products per
    instruction; 4 tensor_tensor ops/step instead of 6.
  - G=2 env-group interleave (columns 0:256 / 256:512): the serial
    recurrence chain of one group hides under the other group's work.
  - critic: wc1 per-step K=64 from the packed [cm';H] tile, tanh stages
    batched [128,*] over 8 steps, bc3 via a ones row, vout DMA'd directly
    from PSUM.
  - fp16 on-chip (more mantissa than bf16 for the sigmoid-trick, same speed).
"""
import numpy as np
import ml_dtypes
from contextlib import ExitStack

import concourse.bass as bass
import concourse.bacc as bacc
import concourse.tile as tile
import concourse.mybir as mybir

F32 = mybir.dt.float32
F16 = mybir.dt.float16
AF = mybir.ActivationFunctionType
ALU = mybir.AluOpType

T_FULL, B_FULL, H = 256, 4096, 64
NCORES = 8
BS = B_FULL // NCORES        # 512 envs per core
GW = BS // 2                 # 256 envs per interleave group
N_WARMUP = 24

float16 = ml_dtypes.float16 if hasattr(ml_dtypes, "float16") else np.float16


def build_module(T=T_FULL, use_bias=False):
    """Per-core Bass module (identical on all cores)."""
    assert T % 8 == 0
    nc = bacc.Bacc("TRN2", target_bir_lowering=False, debug=False)

    # ---- DRAM I/O ----
    xx_d = nc.dram_tensor("xx", [T // 8, 36, 2, BS], F16, kind="ExternalInput")
    mask_d = nc.dram_tensor("maskT", [T + 1, BS], F16, kind="ExternalInput")
    h0_d = nc.dram_tensor("h0m", [64, BS], F16, kind="ExternalInput")
    cm0_d = nc.dram_tensor("cm0", [64, BS], F16, kind="ExternalInput")
    wf_d = nc.dram_tensor("wf", [128, 256], F16, kind="ExternalInput")
    wr1_d = nc.dram_tensor("wr1bd", [36, 128], F16, kind="ExternalInput")
    wr2_d = nc.dram_tensor("wr2bd", [128, 128], F16, kind="ExternalInput")
    wc1_d = nc.dram_tensor("wc1s", [64, 16], F16, kind="ExternalInput")
    wc2_d = nc.dram_tensor("wc2bd", [128, 32], F16, kind="ExternalInput")
    wc3_d = nc.dram_tensor("wc3bd", [33, 4], F16, kind="ExternalInput")
    br1_d = nc.dram_tensor("br1st", [128, 1], F32, kind="ExternalInput")
    br2_d = nc.dram_tensor("br2st", [64, 1], F32, kind="ExternalInput")
    bc1_d = nc.dram_tensor("bc1t", [128, 1], F32, kind="ExternalInput")
    bc2_d = nc.dram_tensor("bc2t", [32, 1], F32, kind="ExternalInput")
    bg0_d = nc.dram_tensor("bg0", [128, 1], F32, kind="ExternalInput")
    bg1_d = nc.dram_tensor("bg1", [128, 1], F32, kind="ExternalInput")
    vout_d = nc.dram_tensor("vout", [T, BS], F32, kind="ExternalOutput")

    RS = 16   # state ring (hid written ~8-11 steps ahead)
    RSG = 3   # sigma-out ring
    RAB = 2   # A/B product ring
    RO = 3    # [cm'; H] / gtilde ring
    RM = 3    # [mask; tanh_c] ring

    with tile.TileContext(nc) as tc, ExitStack() as ctx:
        consts = ctx.enter_context(tc.tile_pool(name="consts", bufs=1))
        persist = ctx.enter_context(tc.tile_pool(name="persist", bufs=1))
        ring = ctx.enter_context(tc.tile_pool(name="ring", bufs=2))
        pp = ctx.enter_context(tc.tile_pool(name="pp", bufs=1, space="PSUM"))

        # ---- constants ----
        wf = consts.tile([128, 256], F16)
        nc.sync.dma_start(wf[:], wf_d[:])
        wr1 = consts.tile([36, 128], F16)
        nc.sync.dma_start(wr1[:], wr1_d[:])
        wr2 = consts.tile([128, 128], F16)
        nc.sync.dma_start(wr2[:], wr2_d[:])
        wc1 = consts.tile([128, 16], F16)   # weights live at rows 64:128
        nc.sync.dma_start(wc1[64:128, :], wc1_d[:])
        wc2 = consts.tile([128, 32], F16)
        nc.sync.dma_start(wc2[:], wc2_d[:])
        wc3 = consts.tile([33, 4], F16)
        nc.sync.dma_start(wc3[:], wc3_d[:])
        br1 = consts.tile([128, 1], F32)
        nc.sync.dma_start(br1[:], br1_d[:])
        br2 = consts.tile([64, 1], F32)
        nc.sync.dma_start(br2[:], br2_d[:])
        bc1t = consts.tile([128, 1], F32)
        nc.sync.dma_start(bc1t[:], bc1_d[:])
        bc2t = consts.tile([32, 1], F32)
        nc.sync.dma_start(bc2t[:], bc2_d[:])
        if use_bias:
            bg0 = consts.tile([128, 1], F32)
            nc.sync.dma_start(bg0[:], bg0_d[:])
            bg1 = consts.tile([128, 1], F32)
            nc.sync.dma_start(bg1[:], bg1_d[:])
        zero64 = consts.tile([64, BS], F16)
        nc.vector.memset(zero64[:], 0.0)
        dummy = consts.tile([128, BS], F16)
        nc.vector.memset(dummy[:], 0.25)

        # ---- persistent ring state ----
        # S: scan matmul rhs.  rows 0:64 hid_t (prepass), 64:128 h_{t-1}
        S = persist.tile([128, RS, BS], F16)
        nc.sync.dma_start(S[64:128, 0, :], h0_d[:])
        # sg: sigma out.  slot0 = [sf; si], slot1 = [u->c'; so]
        sg = persist.tile([128, RSG, 2, BS], F16)
        # ab: A and B products, both on rows 64:128 (slots 0/1 in free dim)
        ab = persist.tile([128, RAB, 2, BS], F16)
        # om: sigma_o * m2 (GPSIMD), rows 64:128
        omt = persist.tile([128, RM, BS], F16)
        # O: [cm'@0:64; H@64:128]; gtilde_{t+1} overwrites 64:128 post-wc1
        O = persist.tile([128, RO, BS], F16)
        nc.sync.dma_start(O[0:64, RO - 1, :], cm0_d[:])
        # mtc: rows 0:64 = mask m_{t+1} (DMA bcast), 64:128 = tanh(c_t)
        mtc = persist.tile([128, RM, BS], F16)
        # msk2: rows 64:128 = mask m_{t+1} (second bcast, base-64 consumers)
        msk2 = persist.tile([128, RM, BS], F16)
        # critic stages (4-step blocks; wc1 writes 16 rows per 32-row slot)
        v1st = persist.tile([128, 2, BS], F16)
        v2st = persist.tile([33, 2, BS], F16)
        nc.vector.memset(v2st[32:33, :, :], 1.0)
        vfin = persist.tile([4, 2, BS], F32)

        # ---- PSUM (8 banks exactly) ----
        pg = pp.tile([128, 2, BS], F32)    # banks 0-1: gates [f;i] | [g2;o]
        pv1 = pp.tile([128, 2, BS], F32)   # banks 2-3: wc1 accum (x8 steps)
        pcv = pp.tile([128, BS], F32)      # bank 4: v2 rows 0:64, wc3 64:72
        pr1 = pp.tile([128, BS], F32)      # bank 5: wr1 out
        pr2 = pp.tile([128, BS], F32)      # bank 6: wr2 out
        pwm = pp.tile([128, BS], F32)      # bank 7: warmup target

        # ---- PE warm-up burst (p-state ramp) ----
        for _ in range(N_WARMUP):
            nc.tensor.matmul(pwm[64:128, :], dummy[:, 0:64], dummy[:],
                             start=True, stop=True, skip_group_check=True)

        def mask_dma(t):
            """broadcast m_{t+1}: mtc rows 0:64 and msk2 rows 64:128"""
            row = mask_d[t + 1:t + 2, :]
            src = bass.AP(tensor=row.tensor, offset=row.offset,
                          ap=[[0, 64], [1, BS]])
            nc.sync.dma_start(mtc[0:64, t % RM, :], src)
            nc.sync.dma_start(msk2[64:128, t % RM, :], src)

        mask_dma(0)
        mask_dma(1)

        # ---------------- prepass (root MLP), 4-step blocks ----------------
        xx_tiles = {}

        def pre_stage(b, j):
            """stage j of 4 for 4-step block b (steps 4b..4b+3)."""
            if j == 0:
                if b % 2 == 0:
                    xxt = ring.tile([36, 2, BS], F16, tag="xx", name=f"xx{b}")
                    nc.sync.dma_start(xxt[:], xx_d[b // 2])
                    xx_tiles[b // 2] = xxt
                nc.tensor.matmul(pr1[:, :], wr1[:], xx_tiles[b // 2][:, b % 2, :],
                                 start=True, stop=True, tile_position=(0, 0))
            elif j == 1:
                h1 = ring.tile([128, BS], F16, tag="h1", name=f"h1_{b}")
                nc.scalar.activation(h1[:], pr1[:, :], AF.Relu, bias=br1[:])
                xx_tiles[f"h1_{b}"] = h1
            else:
                pair = j - 2  # 0 or 1
                h1 = xx_tiles[f"h1_{b}"]
                nc.tensor.matmul(pr2[:, :], wr2[64 * pair:64 * pair + 64, :],
                                 h1[64 * pair:64 * pair + 64, :],
                                 start=True, stop=True,
                                 tile_position=(64 * pair, 0))
                for half in range(2):
                    tt = 4 * b + 2 * pair + half
                    nc.vector.scalar_tensor_tensor(
                        S[0:64, tt % RS, :],
                        pr2[64 * half:64 * half + 64, :],
                        br2[:], zero64[:], ALU.add, ALU.max)
                if pair == 1:
                    del xx_tiles[f"h1_{b}"]

        # ---------------- critic tail stages, 4-step blocks ----------------
        def critic_stage(t):
            """at virtual step t, run stage phi for 4-block cb = t//4 - 1."""
            if t < 4:
                return
            cb = t // 4 - 1
            if cb >= T // 4:
                return
            phi = t % 4
            pb = cb % 2
            if phi == 0:
                for g in range(2):
                    cols = slice(GW * g, GW * g + GW)
                    nc.scalar.activation(v1st[:, pb, cols], pv1[:, pb, cols],
                                         AF.Tanh, bias=bc1t[:])
            elif phi == 1:
                for g in range(2):
                    cols = slice(GW * g, GW * g + GW)
                    nc.tensor.matmul(pcv[0:32, cols], wc2[:],
                                     v1st[:, pb, cols],
                                     start=True, stop=True,
                                     tile_position=(0, 0))
            elif phi == 2:
                for g in range(2):
                    cols = slice(GW * g, GW * g + GW)
                    nc.scalar.activation(v2st[0:32, pb, cols], pcv[0:32, cols],
                                         AF.Tanh, bias=bc2t[:])
            elif phi == 3:
                for g in range(2):
                    cols = slice(GW * g, GW * g + GW)
                    nc.tensor.matmul(pcv[64:68, cols], wc3[:],
                                     v2st[:, pb, cols],
                                     start=True, stop=True,
                                     tile_position=(0, 64))
                nc.vector.tensor_copy(vfin[:, pb, :], pcv[64:68, :])
                nc.sync.dma_start(vout_d[4 * cb:4 * cb + 4, :],
                                  vfin[:, pb, :])

        # prepass warm-up: blocks 0 and 1 fully
        for b in (0, 1):
            for j in range(4):
                pre_stage(b, j)

        LA = 2  # prepass lookahead in 4-step blocks
        n_blocks = T // 4

        # ---------------- main loop ----------------
        for t in range(T):
            s_r = t % RS
            s_w = (t + 1) % RS
            r = t % RSG
            a = t % RAB
            o_r = (t + RO - 1) % RO
            o_w = t % RO
            mr = t % RM

            for g in range(2):
                cols = slice(GW * g, GW * g + GW)
                # -- fused scan matmuls (K=128) --
                nc.tensor.matmul(pg[:, 0, cols], wf[:, 0:128], S[:, s_r, cols],
                                 start=True, stop=True, skip_group_check=True)
                nc.tensor.matmul(pg[:, 1, cols], wf[:, 128:256], S[:, s_r, cols],
                                 start=True, stop=True, skip_group_check=True)
            # -- critic wc1 for step t-1: emitted after this step's matmuls
            # (no head-of-line blocking) but BEFORE gtilde overwrites the H
            # rows it reads (program order = dependency direction) --
            if t > 0:
                q4 = (t - 1) % 4
                qb = ((t - 1) // 4) % 2
                for g in range(2):
                    cols = slice(GW * g, GW * g + GW)
                    nc.tensor.matmul(pv1[32 * q4:32 * q4 + 16, qb, cols],
                                     wc1[64:128, :], O[64:128, o_r, cols],
                                     start=True, stop=True,
                                     tile_position=(64, 32 * q4),
                                     skip_group_check=True)
            for g in range(2):
                cols = slice(GW * g, GW * g + GW)
                # -- all four gates in one strided sigmoid --
                # sg slot0 = [sf@0; si@64], slot1 = [u@0; so@64]
                if use_bias:
                    nc.scalar.activation(sg[:, r, 0, cols], pg[:, 0, cols],
                                         AF.Sigmoid, bias=bg0[:])
                    nc.scalar.activation(sg[:, r, 1, cols], pg[:, 1, cols],
                                         AF.Sigmoid, bias=bg1[:])
                else:
                    nc.scalar.activation(sg[:, r, :, cols], pg[:, :, cols],
                                         AF.Sigmoid)
            for g in range(2):
                cols = slice(GW * g, GW * g + GW)
                # -- gtilde = tanh(g) = 2u - 1 on GPSIMD (u@0 -> out@64) --
                nc.gpsimd.tensor_scalar(O[64:128, o_r, cols],
                                        sg[0:64, r, 1, cols],
                                        2.0, 1.0, ALU.mult, ALU.subtract)
                # -- om = so * m2 on GPSIMD (@64, off the critical path) --
                nc.gpsimd.tensor_tensor(omt[64:128, mr, cols],
                                        sg[64:128, r, 1, cols],
                                        msk2[64:128, mr, cols], ALU.mult)
            for g in range(2):
                cols = slice(GW * g, GW * g + GW)
                # -- B = si * gtilde (both @64) --
                nc.vector.tensor_tensor(ab[64:128, a, 1, cols],
                                        sg[64:128, r, 0, cols],
                                        O[64:128, o_r, cols], ALU.mult)
                # -- A = sf * cm (both @0; out placed @64 next to B) --
                nc.vector.tensor_tensor(ab[64:128, a, 0, cols],
                                        sg[0:64, r, 0, cols],
                                        O[0:64, o_r, cols], ALU.mult)
                # -- c' = A + B (64-64; out over u slot @0) --
                nc.vector.tensor_tensor(sg[0:64, r, 1, cols],
                                        ab[64:128, a, 0, cols],
                                        ab[64:128, a, 1, cols], ALU.add)
            for g in range(2):
                cols = slice(GW * g, GW * g + GW)
                # -- tc = tanh(c') (in @0 -> out @64) --
                nc.scalar.activation(mtc[64:128, mr, cols],
                                     sg[0:64, r, 1, cols], AF.Tanh)
            for g in range(2):
                cols = slice(GW * g, GW * g + GW)
                # -- h_t = om * tc (@64) -> state for t+1 --
                nc.vector.tensor_tensor(S[64:128, s_w, cols],
                                        omt[64:128, mr, cols],
                                        mtc[64:128, mr, cols], ALU.mult)
                # -- [cm'; H] = [c'; so] * [m_{t+1}; tc] (off-chain) --
                nc.vector.tensor_tensor(O[:, o_w, cols], sg[:, r, 1, cols],
                                        mtc[:, mr, cols], ALU.mult)

            # -- lagged off-path work --
            if t + 2 < T:
                mask_dma(t + 2)
            bl = t // 4 + LA
            if bl < n_blocks:
                pre_stage(bl, t % 4)
            critic_stage(t)

        # final wc1 (step T-1), then drain the critic tail
        q4 = (T - 1) % 4
        qb = ((T - 1) // 4) % 2
        for g in range(2):
            cols = slice(GW * g, GW * g + GW)
            nc.tensor.matmul(pv1[32 * q4:32 * q4 + 16, qb, cols],
                             wc1[64:128, :], O[64:128, (T - 1) % RO, cols],
                             start=True, stop=True,
                             tile_position=(64, 32 * q4),
                             skip_group_check=True)
        for t in range(T, T + 8):
            critic_stage(t)

    nc.compile()
    return nc


# ---------------- host-side preparation ----------------

def _prep_core_inputs(inputs, core, T=T_FULL):
    b0, b1 = core * BS, (core + 1) * BS
    x = np.asarray(inputs["x"], np.float32).reshape(T, B_FULL, 9)[:, b0:b1]
    done = np.asarray(inputs["done"]).reshape(T, B_FULL)[:, b0:b1]
    h0 = np.asarray(inputs["h0"], np.float32)[0, b0:b1]  # [BS, 64]
    c0 = np.asarray(inputs["c0"], np.float32)[0, b0:b1]

    donef = done.astype(np.float32)
    maskT = np.ones((T + 1, BS), np.float32)
    maskT[:T] = 1.0 - donef
    h0m = (h0 * maskT[0][:, None]).T        # [64, BS]
    cm0 = (c0 * maskT[0][:, None]).T

    xT = x.transpose(0, 2, 1)  # [T, 9, BS]
    xx = (xT.reshape(T // 8, 2, 4, 9, BS)
            .transpose(0, 2, 3, 1, 4)
            .reshape(T // 8, 36, 2, BS).copy())

    Wih = np.asarray(inputs["Wih"], np.float32)
    Whh = np.asarray(inputs["Whh"], np.float32)
    bl = np.asarray(inputs["b_lstm"], np.float32)
    idx = np.arange(64)
    # stored gate order i,f,g,o -> on-chip col order [f, i, 2*g, o]
    order = np.concatenate([idx + 64, idx, idx + 128, idx + 192])
    wfull = np.concatenate([Wih, Whh], axis=0)[:, order]  # [128, 256]
    wfull[:, 128:192] *= 2.0                              # sigmoid-trick
    bg = bl[order].astype(np.float32)
    bg0 = bg[0:128].copy()
    bg1 = bg[128:256].copy()
    bg1[0:64] *= 2.0
    use_bias = bool(np.any(bl != 0.0))

    Wr1 = np.asarray(inputs["Wr1"], np.float32)
    wr1bd = np.zeros((36, 128), np.float32)
    for k in range(4):
        wr1bd[9 * k:9 * k + 9, 32 * k:32 * k + 32] = Wr1
    Wr2 = np.asarray(inputs["Wr2"], np.float32)
    wr2bd = np.zeros((128, 128), np.float32)
    for half in range(2):
        for j in range(2):
            wr2bd[64 * half + 32 * j:64 * half + 32 * j + 32,
                  64 * j:64 * j + 64] = Wr2

    Wc1 = np.asarray(inputs["Wc1"], np.float32)          # [64, 16]
    Wc2 = np.asarray(inputs["Wc2"], np.float32)          # [16, 8]
    wc2bd = np.zeros((128, 32), np.float32)
    for p in range(4):
        wc2bd[32 * p:32 * p + 16, 8 * p:8 * p + 8] = Wc2
    Wc3 = np.asarray(inputs["Wc3"], np.float32)          # [8, 1]
    bc3 = np.asarray(inputs["bc3"], np.float32)
    wc3bd = np.zeros((33, 4), np.float32)
    for p in range(4):
        wc3bd[8 * p:8 * p + 8, p] = Wc3[:, 0]
    wc3bd[32, :] = bc3[0]

    br1 = np.asarray(inputs["br1"], np.float32)
    br2 = np.asarray(inputs["br2"], np.float32)
    bc1 = np.asarray(inputs["bc1"], np.float32)
    bc2 = np.asarray(inputs["bc2"], np.float32)

    hf = lambda a: np.ascontiguousarray(a).astype(float16)
    f32c = lambda a: np.ascontiguousarray(a, np.float32)
    return {
        "xx": hf(xx), "maskT": hf(maskT), "h0m": hf(h0m), "cm0": hf(cm0),
        "wf": hf(wfull), "wr1bd": hf(wr1bd), "wr2bd": hf(wr2bd),
        "wc1s": hf(Wc1), "wc2bd": hf(wc2bd), "wc3bd": hf(wc3bd),
        "br1st": f32c(np.tile(br1, 4)[:, None]),
        "br2st": f32c(br2[:, None]),
        "bc1t": f32c(np.tile(bc1, 8)[:, None]),
        "bc2t": f32c(np.tile(bc2, 4)[:, None]),
        "bg0": f32c(bg0[:, None]), "bg1": f32c(bg1[:, None]),
    }, use_bias


_NC_CACHE = {}


def _get_module(T=T_FULL, use_bias=False):
    key = (T, use_bias)
    if key not in _NC_CACHE:
        _NC_CACHE[key] = build_module(T, use_bias)
    return _NC_CACHE[key]


def kernel(**inputs) -> np.ndarray:
    from concourse.bass_utils import run_bass_kernel_spmd
    T = T_FULL
    prepped = [_prep_core_inputs(inputs, c, T) for c in range(NCORES)]
    use_bias = any(p[1] for p in prepped)
    in_maps = [p[0] for p in prepped]
    nc = _get_module(T, use_bias)
    res = run_bass_kernel_spmd(nc, in_maps, core_ids=list(range(NCORES)))
    out = np.empty((T, B_FULL), np.float32)
    for c in range(NCORES):
        out[:, c * BS:(c + 1) * BS] = res.results[c]["vout"]
    return out.reshape(T * B_FULL, 1)


# revision 25
# speedup vs baseline: 2.2430x; 2.2430x over previous
"""Trainium2 Bass kernel for nn_Agent_lstm (root MLP -> LSTM scan -> critic).

Data-parallel over B=4096 envs across 8 NeuronCores (512 envs/core), with
feature-on-partition ("transposed") layout on-device.

v2 design (vs. the 4-matmul baseline):
  - fused K=128 scan matmul: state rows = [hid_t (0:64); h_{t-1} (64:128)],
    weights = [Wih; Whh] stacked -> 2 matmul instrs/step instead of 4.
  - sigmoid-only gates: tanh(g) = 2*sigmoid(2g) - 1 with the x2 folded into
    the g weight columns; the (2u-1) fixup runs on the otherwise-idle GPSIMD
    engine.  All four gates then take ONE strided ACTIVATE over both PSUM
    banks per step per group.
  - partition-packed DVE ops: [sf;si]*[cm;gt] -> [A;B] and
    [c';so]*[m;tanh_c] -> [cm';H] compute two 64-row products per
    instruction; 4 tensor_tensor ops/step instead of 6.
  - G=2 env-group interleave (columns 0:256 / 256:512): the serial
    recurrence chain of one group hides under the other group's work.
  - critic: wc1 per-step K=64 from the packed [cm';H] tile, tanh stages
    batched [128,*] over 8 steps, bc3 via a ones row, vout DMA'd directly
    from PSUM.
  - fp16 on-chip (more mantissa than bf16 for the sigmoid-trick, same speed).
"""
import numpy as np
import ml_dtypes
from contextlib import ExitStack

import concourse.bass as bass
import concourse.bacc as bacc
import concourse.tile as tile
import concourse.mybir as mybir

F32 = mybir.dt.float32
F16 = mybir.dt.float16
AF = mybir.ActivationFunctionType
ALU = mybir.AluOpType

T_FULL, B_FULL, H = 256, 4096, 64
NCORES = 8
BS = B_FULL // NCORES        # 512 envs per core
GW = BS // 2                 # 256 envs per interleave group
N_WARMUP = 24

float16 = ml_dtypes.float16 if hasattr(ml_dtypes, "float16") else np.float16


def build_module(T=T_FULL, use_bias=False):
    """Per-core Bass module (identical on all cores)."""
    assert T % 8 == 0
    nc = bacc.Bacc("TRN2", target_bir_lowering=False, debug=False)

    # ---- DRAM I/O ----
    xx_d = nc.dram_tensor("xx", [T // 8, 36, 2, BS], F16, kind="ExternalInput")
    mask_d = nc.dram_tensor("maskT", [T + 1, BS], F16, kind="ExternalInput")
    h0_d = nc.dram_tensor("h0m", [64, BS], F16, kind="ExternalInput")
    cm0_d = nc.dram_tensor("cm0", [64, BS], F16, kind="ExternalInput")
    wf_d = nc.dram_tensor("wf", [128, 256], F16, kind="ExternalInput")
    wr1_d = nc.dram_tensor("wr1bd", [36, 128], F16, kind="ExternalInput")
    wr2_d = nc.dram_tensor("wr2bd", [128, 128], F16, kind="ExternalInput")
    wc1_d = nc.dram_tensor("wc1s", [64, 16], F16, kind="ExternalInput")
    wc2_d = nc.dram_tensor("wc2bd", [128, 32], F16, kind="ExternalInput")
    wc3_d = nc.dram_tensor("wc3bd", [33, 4], F16, kind="ExternalInput")
    br1_d = nc.dram_tensor("br1st", [128, 1], F32, kind="ExternalInput")
    br2_d = nc.dram_tensor("br2st", [64, 1], F32, kind="ExternalInput")
    bc1_d = nc.dram_tensor("bc1t", [128, 1], F32, kind="ExternalInput")
    bc2_d = nc.dram_tensor("bc2t", [32, 1], F32, kind="ExternalInput")
    bg0_d = nc.dram_tensor("bg0", [128, 1], F32, kind="ExternalInput")
    bg1_d = nc.dram_tensor("bg1", [128, 1], F32, kind="ExternalInput")
    vout_d = nc.dram_tensor("vout", [T, BS], F32, kind="ExternalOutput")

    RS = 16   # state ring (hid written ~8-11 steps ahead)
    RSG = 3   # sigma-out ring
    RAB = 2   # A/B product ring
    RO = 3    # [cm'; H] / gtilde ring
    RM = 3    # [mask; tanh_c] ring

    with tile.TileContext(nc) as tc, ExitStack() as ctx:
        consts = ctx.enter_context(tc.tile_pool(name="consts", bufs=1))
        persist = ctx.enter_context(tc.tile_pool(name="persist", bufs=1))
        ring = ctx.enter_context(tc.tile_pool(name="ring", bufs=2))
        pp = ctx.enter_context(tc.tile_pool(name="pp", bufs=1, space="PSUM"))

        # ---- constants ----
        wf = consts.tile([128, 256], F16)
        nc.sync.dma_start(wf[:], wf_d[:])
        wr1 = consts.tile([36, 128], F16)
        nc.sync.dma_start(wr1[:], wr1_d[:])
        wr2 = consts.tile([128, 128], F16)
        nc.sync.dma_start(wr2[:], wr2_d[:])
        wc1 = consts.tile([128, 16], F16)   # weights live at rows 64:128
        nc.sync.dma_start(wc1[64:128, :], wc1_d[:])
        wc2 = consts.tile([128, 32], F16)
        nc.sync.dma_start(wc2[:], wc2_d[:])
        wc3 = consts.tile([33, 4], F16)
        nc.sync.dma_start(wc3[:], wc3_d[:])
        br1 = consts.tile([128, 1], F32)
        nc.sync.dma_start(br1[:], br1_d[:])
        br2 = consts.tile([64, 1], F32)
        nc.sync.dma_start(br2[:], br2_d[:])
        bc1t = consts.tile([128, 1], F32)
        nc.sync.dma_start(bc1t[:], bc1_d[:])
        bc2t = consts.tile([32, 1], F32)
        nc.sync.dma_start(bc2t[:], bc2_d[:])
        if use_bias:
            bg0 = consts.tile([128, 1], F32)
            nc.sync.dma_start(bg0[:], bg0_d[:])
            bg1 = consts.tile([128, 1], F32)
            nc.sync.dma_start(bg1[:], bg1_d[:])
        zero64 = consts.tile([64, BS], F16)
        nc.vector.memset(zero64[:], 0.0)
        dummy = consts.tile([128, BS], F16)
        nc.vector.memset(dummy[:], 0.25)

        # ---- persistent ring state ----
        # S: scan matmul rhs.  rows 0:64 hid_t (prepass), 64:128 h_{t-1}
        S = persist.tile([128, RS, BS], F16)
        nc.sync.dma_start(S[64:128, 0, :], h0_d[:])
        # sg: sigma out.  slot0 = [sf; si], slot1 = [u->c'; so]
        sg = persist.tile([128, RSG, 2, BS], F16)
        # ab: A and B products, both on rows 64:128 (slots 0/1 in free dim)
        ab = persist.tile([128, RAB, 2, BS], F16)
        # O: [cm'@0:64; H@64:128]; gtilde_{t+1} overwrites 64:128 post-wc1
        O = persist.tile([128, RO, BS], F16)
        nc.sync.dma_start(O[0:64, RO - 1, :], cm0_d[:])
        # mtc: rows 0:64 = mask m_{t+1} (DMA bcast), 64:128 = tanh(c_t)
        mtc = persist.tile([128, RM, BS], F16)
        # msk2: rows 64:128 = mask m_{t+1} (second bcast, base-64 consumers)
        msk2 = persist.tile([128, RM, BS], F16)
        # critic stages (4-step blocks; wc1 writes 16 rows per 32-row slot)
        v1st = persist.tile([128, 2, BS], F16)
        v2st = persist.tile([33, 2, BS], F16)
        nc.vector.memset(v2st[32:33, :, :], 1.0)
        vfin = persist.tile([4, 2, BS], F32)

        # ---- PSUM (8 banks exactly) ----
        pg = pp.tile([128, 2, BS], F32)    # banks 0-1: gates [f;i] | [g2;o]
        pv1 = pp.tile([128, 2, BS], F32)   # banks 2-3: wc1 accum (x8 steps)
        pcv = pp.tile([128, BS], F32)      # bank 4: v2 rows 0:64, wc3 64:72
        pr1 = pp.tile([128, BS], F32)      # bank 5: wr1 out
        pr2 = pp.tile([128, BS], F32)      # bank 6: wr2 out
        pwm = pp.tile([128, BS], F32)      # bank 7: warmup target

        # ---- PE warm-up burst (p-state ramp) ----
        for _ in range(N_WARMUP):
            nc.tensor.matmul(pwm[64:128, :], dummy[:, 0:64], dummy[:],
                             start=True, stop=True, skip_group_check=True)

        def mask_dma(t):
            """broadcast m_{t+1}: mtc rows 0:64 and msk2 rows 64:128"""
            row = mask_d[t + 1:t + 2, :]
            src = bass.AP(tensor=row.tensor, offset=row.offset,
                          ap=[[0, 64], [1, BS]])
            nc.sync.dma_start(mtc[0:64, t % RM, :], src)
            nc.sync.dma_start(msk2[64:128, t % RM, :], src)

        mask_dma(0)
        mask_dma(1)

        # ---------------- prepass (root MLP), 4-step blocks ----------------
        xx_tiles = {}

        def pre_stage(b, j):
            """stage j of 4 for 4-step block b (steps 4b..4b+3)."""
            if j == 0:
                if b % 2 == 0:
                    xxt = ring.tile([36, 2, BS], F16, tag="xx", name=f"xx{b}")
                    nc.sync.dma_start(xxt[:], xx_d[b // 2])
                    xx_tiles[b // 2] = xxt
                nc.tensor.matmul(pr1[:, :], wr1[:], xx_tiles[b // 2][:, b % 2, :],
                                 start=True, stop=True, tile_position=(0, 0))
            elif j == 1:
                h1 = ring.tile([128, BS], F16, tag="h1", name=f"h1_{b}")
                nc.scalar.activation(h1[:], pr1[:, :], AF.Relu, bias=br1[:])
                xx_tiles[f"h1_{b}"] = h1
            else:
                pair = j - 2  # 0 or 1
                h1 = xx_tiles[f"h1_{b}"]
                nc.tensor.matmul(pr2[:, :], wr2[64 * pair:64 * pair + 64, :],
                                 h1[64 * pair:64 * pair + 64, :],
                                 start=True, stop=True,
                                 tile_position=(64 * pair, 0))
                for half in range(2):
                    tt = 4 * b + 2 * pair + half
                    nc.scalar.activation(
                        S[0:64, tt % RS, :],
                        pr2[64 * half:64 * half + 64, :],
                        AF.Relu, bias=br2[:])
                if pair == 1:
                    del xx_tiles[f"h1_{b}"]

        # ---------------- critic tail stages, 4-step blocks ----------------
        def critic_stage(t):
            """at virtual step t, run stage phi for 4-block cb = t//4 - 1."""
            if t < 4:
                return
            cb = t // 4 - 1
            if cb >= T // 4:
                return
            phi = t % 4
            pb = cb % 2
            if phi == 0:
                for g in range(2):
                    cols = slice(GW * g, GW * g + GW)
                    nc.scalar.activation(v1st[:, pb, cols], pv1[:, pb, cols],
                                         AF.Tanh, bias=bc1t[:])
            elif phi == 1:
                for g in range(2):
                    cols = slice(GW * g, GW * g + GW)
                    nc.tensor.matmul(pcv[0:32, cols], wc2[:],
                                     v1st[:, pb, cols],
                                     start=True, stop=True,
                                     tile_position=(0, 0))
            elif phi == 2:
                for g in range(2):
                    cols = slice(GW * g, GW * g + GW)
                    nc.scalar.activation(v2st[0:32, pb, cols], pcv[0:32, cols],
                                         AF.Tanh, bias=bc2t[:])
            elif phi == 3:
                for g in range(2):
                    cols = slice(GW * g, GW * g + GW)
                    nc.tensor.matmul(pcv[64:68, cols], wc3[:],
                                     v2st[:, pb, cols],
                                     start=True, stop=True,
                                     tile_position=(0, 64))
                nc.vector.tensor_copy(vfin[:, pb, :], pcv[64:68, :])
                nc.sync.dma_start(vout_d[4 * cb:4 * cb + 4, :],
                                  vfin[:, pb, :])

        # prepass warm-up: blocks 0 and 1 fully
        for b in (0, 1):
            for j in range(4):
                pre_stage(b, j)

        LA = 2  # prepass lookahead in 4-step blocks
        n_blocks = T // 4

        # ---------------- main loop ----------------
        for t in range(T):
            s_r = t % RS
            s_w = (t + 1) % RS
            r = t % RSG
            a = t % RAB
            o_r = (t + RO - 1) % RO
            o_w = t % RO
            mr = t % RM

            for g in range(2):
                cols = slice(GW * g, GW * g + GW)
                # -- fused scan matmuls (K=128) --
                nc.tensor.matmul(pg[:, 0, cols], wf[:, 0:128], S[:, s_r, cols],
                                 start=True, stop=True, skip_group_check=True)
                nc.tensor.matmul(pg[:, 1, cols], wf[:, 128:256], S[:, s_r, cols],
                                 start=True, stop=True, skip_group_check=True)
            # -- critic wc1 for step t-1: emitted after this step's matmuls
            # (no head-of-line blocking) but BEFORE gtilde overwrites the H
            # rows it reads (program order = dependency direction) --
            if t > 0:
                q4 = (t - 1) % 4
                qb = ((t - 1) // 4) % 2
                for g in range(2):
                    cols = slice(GW * g, GW * g + GW)
                    nc.tensor.matmul(pv1[32 * q4:32 * q4 + 16, qb, cols],
                                     wc1[64:128, :], O[64:128, o_r, cols],
                                     start=True, stop=True,
                                     tile_position=(64, 32 * q4),
                                     skip_group_check=True)
            for g in range(2):
                cols = slice(GW * g, GW * g + GW)
                # -- all four gates in one strided sigmoid --
                # sg slot0 = [sf@0; si@64], slot1 = [u@0; so@64]
                if use_bias:
                    nc.scalar.activation(sg[:, r, 0, cols], pg[:, 0, cols],
                                         AF.Sigmoid, bias=bg0[:])
                    nc.scalar.activation(sg[:, r, 1, cols], pg[:, 1, cols],
                                         AF.Sigmoid, bias=bg1[:])
                else:
                    nc.scalar.activation(sg[:, r, :, cols], pg[:, :, cols],
                                         AF.Sigmoid)
            for g in range(2):
                cols = slice(GW * g, GW * g + GW)
                # -- gtilde = tanh(g) = 2u - 1 (u@0 -> out@64, out base free) --
                nc.vector.tensor_scalar(O[64:128, o_r, cols],
                                        sg[0:64, r, 1, cols],
                                        2.0, 1.0, ALU.mult, ALU.subtract)
            for g in range(2):
                cols = slice(GW * g, GW * g + GW)
                # -- B = si * gtilde (both @64) --
                nc.vector.tensor_tensor(ab[64:128, a, 1, cols],
                                        sg[64:128, r, 0, cols],
                                        O[64:128, o_r, cols], ALU.mult)
                # -- A = sf * cm (both @0; out placed @64 next to B) --
                nc.vector.tensor_tensor(ab[64:128, a, 0, cols],
                                        sg[0:64, r, 0, cols],
                                        O[0:64, o_r, cols], ALU.mult)
                # -- c' = A + B (64-64; out over u slot @0) --
                nc.vector.tensor_tensor(sg[0:64, r, 1, cols],
                                        ab[64:128, a, 0, cols],
                                        ab[64:128, a, 1, cols], ALU.add)
            for g in range(2):
                cols = slice(GW * g, GW * g + GW)
                # -- tc = tanh(c') (in @0 -> out @64) --
                nc.scalar.activation(mtc[64:128, mr, cols],
                                     sg[0:64, r, 1, cols], AF.Tanh)
            for g in range(2):
                cols = slice(GW * g, GW * g + GW)
                # -- [cm'; H] = [c'; so] * [m_{t+1}; tc] --
                nc.vector.tensor_tensor(O[:, o_w, cols], sg[:, r, 1, cols],
                                        mtc[:, mr, cols], ALU.mult)
                # -- h_t = H * m2 (both @64) -> state for t+1 --
                nc.vector.tensor_tensor(S[64:128, s_w, cols],
                                        O[64:128, o_w, cols],
                                        msk2[64:128, mr, cols], ALU.mult)

            # -- lagged off-path work --
            if t + 2 < T:
                mask_dma(t + 2)
            bl = t // 4 + LA
            if bl < n_blocks:
                pre_stage(bl, t % 4)
            critic_stage(t)

        # final wc1 (step T-1), then drain the critic tail
        q4 = (T - 1) % 4
        qb = ((T - 1) // 4) % 2
        for g in range(2):
            cols = slice(GW * g, GW * g + GW)
            nc.tensor.matmul(pv1[32 * q4:32 * q4 + 16, qb, cols],
                             wc1[64:128, :], O[64:128, (T - 1) % RO, cols],
                             start=True, stop=True,
                             tile_position=(64, 32 * q4),
                             skip_group_check=True)
        for t in range(T, T + 8):
            critic_stage(t)

    nc.compile()
    return nc


# ---------------- host-side preparation ----------------

def _prep_core_inputs(inputs, core, T=T_FULL):
    b0, b1 = core * BS, (core + 1) * BS
    x = np.asarray(inputs["x"], np.float32).reshape(T, B_FULL, 9)[:, b0:b1]
    done = np.asarray(inputs["done"]).reshape(T, B_FULL)[:, b0:b1]
    h0 = np.asarray(inputs["h0"], np.float32)[0, b0:b1]  # [BS, 64]
    c0 = np.asarray(inputs["c0"], np.float32)[0, b0:b1]

    donef = done.astype(np.float32)
    maskT = np.ones((T + 1, BS), np.float32)
    maskT[:T] = 1.0 - donef
    h0m = (h0 * maskT[0][:, None]).T        # [64, BS]
    cm0 = (c0 * maskT[0][:, None]).T

    xT = x.transpose(0, 2, 1)  # [T, 9, BS]
    xx = (xT.reshape(T // 8, 2, 4, 9, BS)
            .transpose(0, 2, 3, 1, 4)
            .reshape(T // 8, 36, 2, BS).copy())

    Wih = np.asarray(inputs["Wih"], np.float32)
    Whh = np.asarray(inputs["Whh"], np.float32)
    bl = np.asarray(inputs["b_lstm"], np.float32)
    idx = np.arange(64)
    # stored gate order i,f,g,o -> on-chip col order [f, i, 2*g, o]
    order = np.concatenate([idx + 64, idx, idx + 128, idx + 192])
    wfull = np.concatenate([Wih, Whh], axis=0)[:, order]  # [128, 256]
    wfull[:, 128:192] *= 2.0                              # sigmoid-trick
    bg = bl[order].astype(np.float32)
    bg0 = bg[0:128].copy()
    bg1 = bg[128:256].copy()
    bg1[0:64] *= 2.0
    use_bias = bool(np.any(bl != 0.0))

    Wr1 = np.asarray(inputs["Wr1"], np.float32)
    wr1bd = np.zeros((36, 128), np.float32)
    for k in range(4):
        wr1bd[9 * k:9 * k + 9, 32 * k:32 * k + 32] = Wr1
    Wr2 = np.asarray(inputs["Wr2"], np.float32)
    wr2bd = np.zeros((128, 128), np.float32)
    for half in range(2):
        for j in range(2):
            wr2bd[64 * half + 32 * j:64 * half + 32 * j + 32,
                  64 * j:64 * j + 64] = Wr2

    Wc1 = np.asarray(inputs["Wc1"], np.float32)          # [64, 16]
    Wc2 = np.asarray(inputs["Wc2"], np.float32)          # [16, 8]
    wc2bd = np.zeros((128, 32), np.float32)
    for p in range(4):
        wc2bd[32 * p:32 * p + 16, 8 * p:8 * p + 8] = Wc2
    Wc3 = np.asarray(inputs["Wc3"], np.float32)          # [8, 1]
    bc3 = np.asarray(inputs["bc3"], np.float32)
    wc3bd = np.zeros((33, 4), np.float32)
    for p in range(4):
        wc3bd[8 * p:8 * p + 8, p] = Wc3[:, 0]
    wc3bd[32, :] = bc3[0]

    br1 = np.asarray(inputs["br1"], np.float32)
    br2 = np.asarray(inputs["br2"], np.float32)
    bc1 = np.asarray(inputs["bc1"], np.float32)
    bc2 = np.asarray(inputs["bc2"], np.float32)

    hf = lambda a: np.ascontiguousarray(a).astype(float16)
    f32c = lambda a: np.ascontiguousarray(a, np.float32)
    return {
        "xx": hf(xx), "maskT": hf(maskT), "h0m": hf(h0m), "cm0": hf(cm0),
        "wf": hf(wfull), "wr1bd": hf(wr1bd), "wr2bd": hf(wr2bd),
        "wc1s": hf(Wc1), "wc2bd": hf(wc2bd), "wc3bd": hf(wc3bd),
        "br1st": f32c(np.tile(br1, 4)[:, None]),
        "br2st": f32c(br2[:, None]),
        "bc1t": f32c(np.tile(bc1, 8)[:, None]),
        "bc2t": f32c(np.tile(bc2, 4)[:, None]),
        "bg0": f32c(bg0[:, None]), "bg1": f32c(bg1[:, None]),
    }, use_bias


_NC_CACHE = {}


def _get_module(T=T_FULL, use_bias=False):
    key = (T, use_bias)
    if key not in _NC_CACHE:
        _NC_CACHE[key] = build_module(T, use_bias)
    return _NC_CACHE[key]


def kernel(**inputs) -> np.ndarray:
    from concourse.bass_utils import run_bass_kernel_spmd
    T = T_FULL
    prepped = [_prep_core_inputs(inputs, c, T) for c in range(NCORES)]
    use_bias = any(p[1] for p in prepped)
    in_maps = [p[0] for p in prepped]
    nc = _get_module(T, use_bias)
    res = run_bass_kernel_spmd(nc, in_maps, core_ids=list(range(NCORES)))
    out = np.empty((T, B_FULL), np.float32)
    for c in range(NCORES):
        out[:, c * BS:(c + 1) * BS] = res.results[c]["vout"]
    return out.reshape(T * B_FULL, 1)
